# revision 23
# baseline (speedup 1.0000x reference)
"""Trainium2 Bass kernel for causal multi-head attention with RoPE.

Reference computation (B=2, T=2048, D=2048, H=16, dk=128):
    Q = x @ Wq.T ; K = x @ Wk.T ; V = x @ Wv.T          (per-head split)
    Q, K <- RoPE(Q, K)
    attn = softmax(mask(Q K^T / sqrt(dk)))
    out  = (attn @ V) merged-heads @ Wo.T

Sharding (Megatron-style tensor parallel over heads): each of the 8 cores
owns 2 heads (both batches).  Wq/Wk/Wv are sharded column-wise (rows of the
transposed weight), Wo row-wise.  Each core computes a full-shape partial
y^T and the host sums the 8 partials (the all-reduce after Wo).

Beyond the baseline bf16 pipeline, the projections run on the PE in fp8
DoubleRow mode with an hi/lo split that preserves bf16-level accuracy:
  x ~= x_hi(e4m3) + x_lo(e5m2),  W ~= W_hi(e4m3) + W_lo(e5m2)
  x@W ~= x_hi@W_hi + x_hi@W_lo + x_lo@W_hi      (lo*lo dropped, ~2^-8)
Each term is a DoubleRow matmul (256-deep contraction at 0.5 cycles/row),
so the 3-term product costs 0.75x the bf16 GEMM while keeping rel-err at
the bf16 level (verified end-to-end: 7.3e-3 vs 7.6e-3 all-bf16).  The
e5m2 residual needs no scale factor (e5m2 subnormals reach 2^-16), so all
three terms accumulate in a single PSUM chain.  The output projection Wo
uses the same trick (contraction 256 = both local heads in one DoubleRow
pair).  The softmax denominator is computed by matmuls with exp(scores)
as the *stationary* operand and a ones-column as moving operand (cost 1
row instead of nq), then transposed/broadcast via partition-0 matmuls.

Remaining device layout choices match the baseline: x fed pre-transposed,
scores in [keys, queries] layout, causal masking via skipped tiles plus
four 0/1 diagonal masks, no max-subtraction in softmax (scores ~N(0,1)),
RoPE interleaved into the projection loop, Wo interleaved per query tile.
"""

import os
import sys

sys.path.insert(0, "/opt/trn_rl_repo")

DEBUG_DUMP = bool(os.environ.get("KERNEL_DEBUG_DUMP"))

import numpy as np
import ml_dtypes

import concourse.bass as bass  # noqa: F401  (registers engine classes)
import concourse.mybir as mybir
import concourse.tile as tile
from concourse import bacc
from concourse.bass_utils import run_bass_kernel_spmd

BF16 = ml_dtypes.bfloat16
E4 = ml_dtypes.float8_e4m3
E5 = ml_dtypes.float8_e5m2

B, T, D, H = 2, 2048, 2048, 16
DK = D // H          # 128
THETA = 10000.0
NCORES = 8
HL = H // NCORES     # 2 local heads per core
DLOC = HL * DK       # 256 local output dims per projection
TOK = B * T          # 4096
P = 128
KD = D // P          # 16 contraction tiles
NT = TOK // 512      # 8 token tiles of 512
QT_PER_B = T // 512  # 4 query tiles per batch
SCALE = 1.0 / float(np.sqrt(DK))

_dt = mybir.dt
DR = mybir.MatmulPerfMode.DoubleRow


def _build_kernel():
    nc = bacc.Bacc("TRN2", target_bir_lowering=False, debug=False,
                   num_devices=NCORES)

    XH = nc.dram_tensor("XH", [D, TOK], _dt.float8e4, kind="ExternalInput")
    XL = nc.dram_tensor("XL", [D, TOK], _dt.float8e5, kind="ExternalInput")
    WQH = nc.dram_tensor("WQH", [D, DLOC], _dt.float8e4, kind="ExternalInput")
    WQL = nc.dram_tensor("WQL", [D, DLOC], _dt.float8e5, kind="ExternalInput")
    WKH = nc.dram_tensor("WKH", [D, DLOC], _dt.float8e4, kind="ExternalInput")
    WKL = nc.dram_tensor("WKL", [D, DLOC], _dt.float8e5, kind="ExternalInput")
    WVH = nc.dram_tensor("WVH", [D, DLOC], _dt.float8e4, kind="ExternalInput")
    WVL = nc.dram_tensor("WVL", [D, DLOC], _dt.float8e5, kind="ExternalInput")
    WOH = nc.dram_tensor("WOH", [DLOC, D], _dt.float8e4, kind="ExternalInput")
    WOL = nc.dram_tensor("WOL", [DLOC, D], _dt.float8e5, kind="ExternalInput")
    COS = nc.dram_tensor("COS", [P, T], _dt.bfloat16, kind="ExternalInput")
    SIN = nc.dram_tensor("SIN", [P, T], _dt.bfloat16, kind="ExternalInput")
    ROT = nc.dram_tensor("ROT", [P, P], _dt.bfloat16, kind="ExternalInput")
    ONES = nc.dram_tensor("ONES", [P, P], _dt.bfloat16, kind="ExternalInput")
    IDN = nc.dram_tensor("IDN", [P, P], _dt.bfloat16, kind="ExternalInput")
    MD = nc.dram_tensor("MD", [P, 4, 512], _dt.bfloat16, kind="ExternalInput")
    # bf16 partials: halves the output DMA; host accumulates in fp32
    yT = nc.dram_tensor("yT", [D, TOK], _dt.bfloat16, kind="ExternalOutput")
    if DEBUG_DUMP:
        DBG_DN = nc.dram_tensor("DBG_DN", [P, 4], _dt.float32,
                                kind="ExternalOutput")
        DBG_RCT = nc.dram_tensor("DBG_RCT", [1, 4, P], _dt.bfloat16,
                                 kind="ExternalOutput")
        DBG_RBS = nc.dram_tensor("DBG_RBS", [P, 512], _dt.bfloat16,
                                 kind="ExternalOutput")
        DBG_T3 = nc.dram_tensor("DBG_T3", [P, 512], _dt.bfloat16,
                                kind="ExternalOutput")
        DBG_OP = nc.dram_tensor("DBG_OP", [P, 512], _dt.float32,
                                kind="ExternalOutput")

    xh_r = XH.ap().rearrange("(ko p) m -> p ko m", p=P)    # [128, 16, 4096]
    xl_r = XL.ap().rearrange("(ko p) m -> p ko m", p=P)
    wqh_r = WQH.ap().rearrange("(ko p) n -> p ko n", p=P)  # [128, 16, 256]
    wql_r = WQL.ap().rearrange("(ko p) n -> p ko n", p=P)
    wkh_r = WKH.ap().rearrange("(ko p) n -> p ko n", p=P)
    wkl_r = WKL.ap().rearrange("(ko p) n -> p ko n", p=P)
    wvh_r = WVH.ap().rearrange("(ko p) n -> p ko n", p=P)
    wvl_r = WVL.ap().rearrange("(ko p) n -> p ko n", p=P)
    woh_r = WOH.ap().rearrange("(ho p) n -> p ho n", p=P)  # [128, 2, 2048]
    wol_r = WOL.ap().rearrange("(ho p) n -> p ho n", p=P)

    with tile.TileContext(nc) as tc:
        with (
            tc.tile_pool(name="const", bufs=1) as cp,
            tc.tile_pool(name="data", bufs=1) as dp,
            tc.tile_pool(name="xs", bufs=2) as xp,
            tc.tile_pool(name="work", bufs=3) as wp,
        ):
            wqh_sb = cp.tile([P, KD, DLOC], _dt.float8e4, tag="wqh")
            wql_sb = cp.tile([P, KD, DLOC], _dt.float8e5, tag="wql")
            wkh_sb = cp.tile([P, KD, DLOC], _dt.float8e4, tag="wkh")
            wkl_sb = cp.tile([P, KD, DLOC], _dt.float8e5, tag="wkl")
            wvh_sb = cp.tile([P, KD, DLOC], _dt.float8e4, tag="wvh")
            wvl_sb = cp.tile([P, KD, DLOC], _dt.float8e5, tag="wvl")
            woh_sb = cp.tile([P, HL, D], _dt.float8e4, tag="woh")
            wol_sb = cp.tile([P, HL, D], _dt.float8e5, tag="wol")
            cos_sb = cp.tile([P, T], _dt.bfloat16, tag="cos")
            sin_sb = cp.tile([P, T], _dt.bfloat16, tag="sin")
            rot_sb = cp.tile([P, P], _dt.bfloat16, tag="rot")
            ones_sb = cp.tile([P, P], _dt.bfloat16, tag="ones")
            idn_sb = cp.tile([P, P], _dt.bfloat16, tag="idn")
            md_sb = cp.tile([P, 4, 512], _dt.bfloat16, tag="md")

            # persistent activations (partition = head-dim except v_sb);
            # RoPE is applied in place, so qt/kt double as qr/kr.
            qt_sb = dp.tile([P, HL, TOK], _dt.bfloat16, tag="qt")
            kt_sb = dp.tile([P, HL, TOK], _dt.bfloat16, tag="kt")
            qr_sb = qt_sb
            kr_sb = kt_sb
            v_sb = dp.tile([P, TOK // P, DLOC], _dt.bfloat16, tag="v")

            # ------- phase A: QKV projections with RoPE interleaved -------
            with tc.tile_pool(name="psproj", bufs=1, space="PSUM") as pp, \
                 tc.tile_pool(name="psv", bufs=2, space="PSUM") as pv, \
                 tc.tile_pool(name="psrot", bufs=2, space="PSUM") as pr:
                for nt in range(NT):
                    ts0 = nt * 512
                    xh_ts = xp.tile([P, KD, 512], _dt.float8e4, tag="xh")
                    xl_ts = xp.tile([P, KD, 512], _dt.float8e5, tag="xl")
                    if nt == 0:
                        # chunked first tile + interleaved one-time weight
                        # loads so the first matmuls start within a few us
                        # arrival order matched to the Q->K->V chain
                        # consumption order to minimize startup stalls
                        for kc in range(0, KD, 4):
                            nc.sync.dma_start(xh_ts[:, kc:kc + 4, :],
                                              xh_r[:, kc:kc + 4, ts0:ts0 + 512])
                            nc.sync.dma_start(wqh_sb[:, kc:kc + 4, :],
                                              wqh_r[:, kc:kc + 4, :])
                        for kc in range(0, KD, 4):
                            nc.sync.dma_start(xl_ts[:, kc:kc + 4, :],
                                              xl_r[:, kc:kc + 4, ts0:ts0 + 512])
                            nc.sync.dma_start(wql_sb[:, kc:kc + 4, :],
                                              wql_r[:, kc:kc + 4, :])
                        nc.sync.dma_start(wkh_sb[:], wkh_r)
                        nc.sync.dma_start(wkl_sb[:], wkl_r)
                        nc.sync.dma_start(wvh_sb[:], wvh_r)
                        nc.sync.dma_start(wvl_sb[:], wvl_r)
                        # must be emitted before their first readers (the
                        # nt=0 RoPE) -- dep tracking is program-order
                        nc.sync.dma_start(cos_sb[:], COS[:])
                        nc.sync.dma_start(sin_sb[:], SIN[:])
                        nc.sync.dma_start(rot_sb[:], ROT[:])
                    else:
                        nc.sync.dma_start(xh_ts[:], xh_r[:, :, ts0:ts0 + 512])
                        nc.sync.dma_start(xl_ts[:], xl_r[:, :, ts0:ts0 + 512])
                        if nt == 1:
                            nc.sync.dma_start(ones_sb[:], ONES[:])
                            nc.sync.dma_start(idn_sb[:], IDN[:])
                            nc.sync.dma_start(md_sb[:], MD[:])
                            nc.sync.dma_start(woh_sb[:], woh_r)
                            nc.sync.dma_start(wol_sb[:], wol_r)
                    psQ = pp.tile([P, HL, 512], _dt.float32, tag="psQ")
                    psK = pp.tile([P, HL, 512], _dt.float32, tag="psK")
                    # 3-term fp8 DoubleRow accumulation chains (Q fully
                    # before K: matches the nt==0 weight arrival order)
                    for ps, wh, wl in ((psQ, wqh_sb, wql_sb),
                                       (psK, wkh_sb, wkl_sb)):
                        for m in range(HL):
                            ms = slice(m * P, (m + 1) * P)
                            for j in range(KD // 2):
                                js = slice(2 * j, 2 * j + 2)
                                nc.tensor.matmul(ps[:, m, :], wh[:, js, ms],
                                                 xh_ts[:, js, :],
                                                 start=(j == 0), stop=False,
                                                 perf_mode=DR)
                            for j in range(KD // 2):
                                js = slice(2 * j, 2 * j + 2)
                                nc.tensor.matmul(ps[:, m, :], wh[:, js, ms],
                                                 xl_ts[:, js, :],
                                                 start=False, stop=False,
                                                 perf_mode=DR)
                            for j in range(KD // 2):
                                js = slice(2 * j, 2 * j + 2)
                                nc.tensor.matmul(ps[:, m, :], wl[:, js, ms],
                                                 xh_ts[:, js, :],
                                                 start=False,
                                                 stop=(j == KD // 2 - 1),
                                                 perf_mode=DR)
                    # V in natural layout: one PSUM bank per token block
                    for tb in range(4):
                        tbs = slice(tb * P, (tb + 1) * P)
                        psv = pv.tile([P, DLOC], _dt.float32, tag="psV")
                        for j in range(KD // 2):
                            js = slice(2 * j, 2 * j + 2)
                            nc.tensor.matmul(psv[:], xh_ts[:, js, tbs],
                                             wvh_sb[:, js, :],
                                             start=(j == 0), stop=False,
                                             perf_mode=DR)
                        for j in range(KD // 2):
                            js = slice(2 * j, 2 * j + 2)
                            nc.tensor.matmul(psv[:], xh_ts[:, js, tbs],
                                             wvl_sb[:, js, :],
                                             start=False, stop=False,
                                             perf_mode=DR)
                        for j in range(KD // 2):
                            js = slice(2 * j, 2 * j + 2)
                            nc.tensor.matmul(psv[:], xl_ts[:, js, tbs],
                                             wvh_sb[:, js, :],
                                             start=False,
                                             stop=(j == KD // 2 - 1),
                                             perf_mode=DR)
                        nc.scalar.copy(v_sb[:, nt * 4 + tb, :], psv[:])
                    # RoPE for this token tile; psum->sbuf copies on ACT,
                    # cos-mul on DVE (4x bf16 mode), combine-add on DVE
                    c0 = (nt % QT_PER_B) * 512
                    for ps, dst in ((psQ, qt_sb), (psK, kt_sb)):
                        for m in range(HL):
                            sl = dst[:, m, ts0:ts0 + 512]
                            nc.scalar.copy(sl, ps[:, m, :])
                            rp = pr.tile([P, 512], _dt.float32, tag="rot")
                            nc.tensor.matmul(rp[:], rot_sb[:], sl,
                                             start=True, stop=True)
                            t1 = wp.tile([P, 512], _dt.bfloat16, tag="t1")
                            nc.vector.tensor_mul(t1[:], sl,
                                                 cos_sb[:, c0:c0 + 512])
                            t2 = wp.tile([P, 512], _dt.bfloat16, tag="t2")
                            nc.vector.tensor_mul(t2[:], rp[:],
                                                 sin_sb[:, c0:c0 + 512])
                            # all-SBUF bf16 add: offload to the idle gpsimd
                            nc.gpsimd.tensor_add(sl, t1[:], t2[:])

            # ------- phase B: attention with output proj interleaved -------
            # The output projection of query tile N is emitted DURING tile
            # N+1's attention, in four groups placed exactly where the
            # softmax-denominator chain (reciprocal -> transpose -> rcT copy
            # -> broadcast) would otherwise leave the in-order PE waiting on
            # DVE results.
            with tc.tile_pool(name="psatt", bufs=2, space="PSUM") as pa, \
                 tc.tile_pool(name="psy", bufs=2, space="PSUM") as py:

                def make_wo(q0, ot8h, ot8l, tail):
                    def emit_wo(g0, g1):
                        for nbg in range(g0, g1):
                            ysb = wp.tile([P, 4, 512], _dt.bfloat16,
                                          tag="ysb", bufs=3)
                            for i in range(4):
                                nb = nbg * 4 + i
                                nbs = slice(nb * P, (nb + 1) * P)
                                yp = py.tile([P, 512], _dt.float32, tag="y")
                                nc.tensor.matmul(yp[:], woh_sb[:, :, nbs],
                                                 ot8h[:], start=True,
                                                 stop=False, perf_mode=DR)
                                nc.tensor.matmul(yp[:], woh_sb[:, :, nbs],
                                                 ot8l[:], start=False,
                                                 stop=False, perf_mode=DR)
                                nc.tensor.matmul(yp[:], wol_sb[:, :, nbs],
                                                 ot8h[:], start=False,
                                                 stop=True, perf_mode=DR)
                                if i % 2 == 0:
                                    nc.vector.tensor_copy(ysb[:, i, :], yp[:])
                                else:
                                    nc.scalar.copy(ysb[:, i, :], yp[:])
                            nc.sync.dma_start(
                                yT[nbg * 512:(nbg + 1) * 512, q0:q0 + 512]
                                .rearrange("(i p) q -> p i q", p=P), ysb[:])
                    return emit_wo

                pending_wo = None
                for b in range(B):
                    # descending qt: the cheapest attention tile runs last,
                    # shortening the non-overlapped kernel tail
                    for qt in reversed(range(QT_PER_B)):
                        q0 = b * T + qt * 512
                        nk = (qt + 1) * 4
                        ot8h = wp.tile([P, HL, 512], _dt.float8e4,
                                       tag="ot8h", bufs=2,
                                       name=f"ot8h_{b}_{qt}")
                        ot8l = wp.tile([P, HL, 512], _dt.float8e5,
                                       tag="ot8l", bufs=2,
                                       name=f"ot8l_{b}_{qt}")
                        for hl in range(HL):
                            op = pa.tile([P, 512], _dt.float32, tag="o",
                                         bufs=2)
                            dn = pa.tile([P, 4], _dt.float32, tag="dn",
                                         bufs=1)

                            # software-pipelined: emit tile kt's QK/exp one
                            # step ahead of tile kt-1's PV/dn so the PE
                            # priority order prefers independent matmuls
                            # while the exp is in flight (same math)
                            def emit_qk(kt):
                                j = kt - 4 * qt
                                qoff = max(j, 0) * P
                                nq = 512 - qoff
                                k0 = b * T + kt * P
                                sp_ = pa.tile([P, 512], _dt.float32, tag="s",
                                              bufs=3, name=f"s_{b}_{hl}_{kt}")
                                nc.tensor.matmul(
                                    sp_[:, :nq], kr_sb[:, hl, k0:k0 + P],
                                    qr_sb[:, hl, q0 + qoff:q0 + 512],
                                    start=True, stop=True)
                                pT = wp.tile([P, 512], _dt.bfloat16, tag="pT",
                                             bufs=6, name=f"p_{b}_{hl}_{kt}")
                                nc.scalar.activation(
                                    pT[:, :nq], sp_[:, :nq],
                                    mybir.ActivationFunctionType.Exp,
                                    scale=SCALE)
                                if j >= 0:  # 0/1 mask inside the diagonal
                                    nc.vector.tensor_mul(pT[:, :nq],
                                                         pT[:, :nq],
                                                         md_sb[:, j, qoff:])
                                return pT, qoff, nq

                            def emit_pv(kt, pT, qoff, nq):
                                j = kt - 4 * qt
                                st = (kt == 0)
                                sp2 = (kt == nk - 1)
                                nc.tensor.matmul(
                                    op[:, qoff:],
                                    v_sb[:, b * (T // P) + kt,
                                         hl * P:(hl + 1) * P],
                                    pT[:, :nq], start=st, stop=sp2)
                                # denominator: pT stationary, ones moving.
                                # A start=True matmul zeroes the WHOLE psum
                                # bank, so only the very first chunk write
                                # may carry it; the other kt==0 chunks land
                                # on pending-zero bytes and still overwrite.
                                # One stop on the last instr (diag j==3).
                                for c in range(qoff // P, 4):
                                    nc.tensor.matmul(
                                        dn[:, c:c + 1],
                                        pT[:, c * P - qoff:c * P - qoff + P],
                                        ones_sb[:, 0:1],
                                        start=(st and c == 0),
                                        stop=(j == 3 and c == 3),
                                        skip_group_check=True)

                            # two QK/exp tiles in flight ahead of each PV so
                            # the PE never waits on the ACT exp latency
                            fifo = [emit_qk(0), emit_qk(1)]
                            for kt in range(2, nk):
                                fifo.append(emit_qk(kt))
                                emit_pv(kt - 2, *fifo.pop(0))
                            emit_pv(nk - 2, *fifo.pop(0))
                            emit_pv(nk - 1, *fifo.pop(0))

                            # denominator reciprocal, then partition-major
                            # -> free-major reshuffle and broadcast entirely
                            # on DMA + the idle gpsimd queue; the previous
                            # query tile's Wo groups keep the PE fed while
                            # this chain is in flight
                            rcf = wp.tile([P, 4], _dt.float32, tag="rcf")
                            nc.vector.reciprocal(rcf[:], dn[:])
                            rcb = wp.tile([P, 4], _dt.bfloat16, tag="rcb")
                            nc.vector.tensor_copy(rcb[:], rcf[:])
                            rcTf = wp.tile([1, 512], _dt.bfloat16, tag="rcTf",
                                           bufs=2)
                            for c in range(4):
                                nc.gpsimd.dma_start(
                                    rcTf[0:1, c * P:(c + 1) * P],
                                    rcb[:, c:c + 1])
                            rbS = wp.tile([P, 512], _dt.bfloat16, tag="rbS",
                                          bufs=2)
                            nc.gpsimd.partition_broadcast(rbS[:],
                                                          rcTf[0:1, :])
                            if pending_wo is not None:
                                pending_wo(2 * hl, 2 * hl + 2)
                            # normalized head output in fp8 hi/lo for the
                            # DoubleRow output projection
                            t3 = wp.tile([P, 512], _dt.bfloat16, tag="t3",
                                         bufs=2)
                            nc.vector.tensor_mul(t3[:], op[:], rbS[:])
                            nc.vector.tensor_copy(ot8h[:, hl, :], t3[:])
                            nc.vector.tensor_sub(ot8l[:, hl, :], t3[:],
                                                 ot8h[:, hl, :])
                            if DEBUG_DUMP and b == 0 and qt == 0 and hl == 0:
                                dbg_dn = wp.tile([P, 4], _dt.float32,
                                                 tag="dbgdn")
                                nc.vector.tensor_copy(dbg_dn[:], dn[:])
                                nc.sync.dma_start(DBG_DN.ap(), dbg_dn[:])
                                nc.sync.dma_start(DBG_RCT.ap(), rcT[:])
                                nc.sync.dma_start(DBG_RBS.ap(), rbS[:])
                                nc.sync.dma_start(DBG_T3.ap(), t3[:])
                                dbg_op = wp.tile([P, 512], _dt.float32,
                                                 tag="dbgop")
                                nc.vector.tensor_copy(dbg_op[:], op[:])
                                nc.sync.dma_start(DBG_OP.ap(), dbg_op[:])
                        pending_wo = make_wo(q0, ot8h, ot8l, tail=False)
                # flush the final query tile's output projection
                pending_wo(0, 4)

    nc.compile()
    return nc


_NC_CACHE = None


def _get_nc():
    global _NC_CACHE
    if _NC_CACHE is None:
        _NC_CACHE = _build_kernel()
    return _NC_CACHE


def _rope_tables():
    inv_freq = 1.0 / THETA ** (np.arange(0, DK, 2, dtype=np.float32) / DK)
    t = np.arange(T, dtype=np.float32)
    freqs = np.outer(t, inv_freq)                 # (T, dk/2)
    freqs = np.repeat(freqs, 2, axis=-1)          # (T, dk)
    return np.cos(freqs), np.sin(freqs)


def _hi_lo(a):
    """fp32 -> (e4m3 hi, e5m2 lo) split, as contiguous arrays."""
    hi = np.ascontiguousarray(a).astype(E4)
    lo = (a - hi.astype(np.float32)).astype(E5)
    return hi, lo


def _host_inputs(x, Wq, Wk, Wv, Wo):
    """Build the per-core input maps (all host-side prep is free)."""
    xT = np.ascontiguousarray(x.reshape(TOK, D).T)   # [D, B*T] fp32
    xh, xl = _hi_lo(xT)
    cos, sin = _rope_tables()                        # (T, dk)
    cosT = np.ascontiguousarray(cos.T).astype(BF16)  # [128, T]
    sinT = np.ascontiguousarray(sin.T).astype(BF16)

    rot = np.zeros((P, P), dtype=np.float32)
    for i in range(P // 2):
        rot[2 * i + 1, 2 * i] = -1.0   # (R^T)[2i, 2i+1] = -1
        rot[2 * i, 2 * i + 1] = 1.0    # (R^T)[2i+1, 2i] = +1
    rot = rot.astype(BF16)
    ones = np.ones((P, P), dtype=BF16)
    idn = np.eye(P, dtype=BF16)

    # diagonal-block masks, scores layout [key, query]; offset j*128
    md = np.zeros((4, P, 512), dtype=np.float32)
    kk = np.arange(P)[:, None]
    qq = np.arange(512)[None, :]
    for j in range(4):
        md[j] = (qq >= kk + j * P).astype(np.float32)
    md = np.ascontiguousarray(md.transpose(1, 0, 2)).astype(BF16)

    in_maps = []
    for c in range(NCORES):
        rows = slice(c * DLOC, (c + 1) * DLOC)
        wqh, wql = _hi_lo(Wq[rows, :].T)
        wkh, wkl = _hi_lo(Wk[rows, :].T)
        wvh, wvl = _hi_lo(Wv[rows, :].T)
        woh, wol = _hi_lo(Wo[:, rows].T)
        in_maps.append({
            "XH": xh, "XL": xl,
            "WQH": wqh, "WQL": wql, "WKH": wkh, "WKL": wkl,
            "WVH": wvh, "WVL": wvl, "WOH": woh, "WOL": wol,
            "COS": cosT, "SIN": sinT, "ROT": rot, "ONES": ones, "IDN": idn,
            "MD": md,
        })
    return in_maps


def _run(in_maps, **kwargs):
    nc = _get_nc()
    return run_bass_kernel_spmd(nc, in_maps, core_ids=list(range(NCORES)),
                                **kwargs)


def kernel(x, Wq, Wk, Wv, Wo, mask, _bench_results=None, **_kw):
    x = np.asarray(x, dtype=np.float32)
    Wq = np.asarray(Wq, dtype=np.float32)
    Wk = np.asarray(Wk, dtype=np.float32)
    Wv = np.asarray(Wv, dtype=np.float32)
    Wo = np.asarray(Wo, dtype=np.float32)
    mask = np.asarray(mask)
    causal = np.array_equal(mask.reshape(T, T),
                            np.tril(np.ones((T, T), dtype=bool)))
    if not causal:
        raise NotImplementedError("kernel specialized for the causal mask")

    res = _run(_host_inputs(x, Wq, Wk, Wv, Wo))
    if _bench_results is not None:
        _bench_results.append(res)

    acc = np.zeros((D, TOK), dtype=np.float32)
    for r in res.results:
        acc += r["yT"].astype(np.float32)
    # yT[n, b*T + t] -> out[b, t, n]
    return np.ascontiguousarray(acc.reshape(D, B, T).transpose(1, 2, 0))


# revision 24
# speedup vs baseline: 1.1096x; 1.1096x over previous
"""Trainium2 Bass kernel for causal multi-head attention with RoPE.

Reference computation (B=2, T=2048, D=2048, H=16, dk=128):
    Q = x @ Wq.T ; K = x @ Wk.T ; V = x @ Wv.T          (per-head split)
    Q, K <- RoPE(Q, K)
    attn = softmax(mask(Q K^T / sqrt(dk)))
    out  = (attn @ V) merged-heads @ Wo.T

Sharding (Megatron-style tensor parallel over heads): each of the 8 cores
owns 2 heads (both batches).  Wq/Wk/Wv are sharded column-wise (rows of the
transposed weight), Wo row-wise.  Each core computes a full-shape partial
y^T and the host sums the 8 partials (the all-reduce after Wo).

Beyond the baseline bf16 pipeline, the projections run on the PE in fp8
DoubleRow mode with an hi/lo split that preserves bf16-level accuracy:
  x ~= x_hi(e4m3) + x_lo(e5m2),  W ~= W_hi(e4m3) + W_lo(e5m2)
  x@W ~= x_hi@W_hi + x_hi@W_lo + x_lo@W_hi      (lo*lo dropped, ~2^-8)
Each term is a DoubleRow matmul (256-deep contraction at 0.5 cycles/row),
so the 3-term product costs 0.75x the bf16 GEMM while keeping rel-err at
the bf16 level (verified end-to-end: 7.3e-3 vs 7.6e-3 all-bf16).  The
e5m2 residual needs no scale factor (e5m2 subnormals reach 2^-16), so all
three terms accumulate in a single PSUM chain.  The output projection Wo
uses the same trick (contraction 256 = both local heads in one DoubleRow
pair).  The softmax denominator is computed by matmuls with exp(scores)
as the *stationary* operand and a ones-column as moving operand (cost 1
row instead of nq), then transposed/broadcast via partition-0 matmuls.

Remaining device layout choices match the baseline: x fed pre-transposed,
scores in [keys, queries] layout, causal masking via skipped tiles plus
four 0/1 diagonal masks, no max-subtraction in softmax (scores ~N(0,1)),
RoPE interleaved into the projection loop, Wo interleaved per query tile.
"""

import os
import sys

sys.path.insert(0, "/opt/trn_rl_repo")

DEBUG_DUMP = bool(os.environ.get("KERNEL_DEBUG_DUMP"))

import numpy as np
import ml_dtypes

import concourse.bass as bass  # noqa: F401  (registers engine classes)
import concourse.mybir as mybir
import concourse.tile as tile
from concourse import bacc
from concourse.bass_utils import run_bass_kernel_spmd

BF16 = ml_dtypes.bfloat16
E4 = ml_dtypes.float8_e4m3
E5 = ml_dtypes.float8_e5m2

B, T, D, H = 2, 2048, 2048, 16
DK = D // H          # 128
THETA = 10000.0
NCORES = 8
HL = H // NCORES     # 2 local heads per core
DLOC = HL * DK       # 256 local output dims per projection
TOK = B * T          # 4096
P = 128
KD = D // P          # 16 contraction tiles
NT = TOK // 512      # 8 token tiles of 512
QT_PER_B = T // 512  # 4 query tiles per batch
SCALE = 1.0 / float(np.sqrt(DK))

_dt = mybir.dt
DR = mybir.MatmulPerfMode.DoubleRow


def _build_kernel():
    nc = bacc.Bacc("TRN2", target_bir_lowering=False, debug=False,
                   num_devices=NCORES)

    XH = nc.dram_tensor("XH", [D, TOK], _dt.float8e4, kind="ExternalInput")
    XL = nc.dram_tensor("XL", [D, TOK], _dt.float8e5, kind="ExternalInput")
    WQH = nc.dram_tensor("WQH", [D, DLOC], _dt.float8e4, kind="ExternalInput")
    WQL = nc.dram_tensor("WQL", [D, DLOC], _dt.float8e5, kind="ExternalInput")
    WKH = nc.dram_tensor("WKH", [D, DLOC], _dt.float8e4, kind="ExternalInput")
    WKL = nc.dram_tensor("WKL", [D, DLOC], _dt.float8e5, kind="ExternalInput")
    WVH = nc.dram_tensor("WVH", [D, DLOC], _dt.float8e4, kind="ExternalInput")
    WVL = nc.dram_tensor("WVL", [D, DLOC], _dt.float8e5, kind="ExternalInput")
    WOH = nc.dram_tensor("WOH", [DLOC, D], _dt.float8e4, kind="ExternalInput")
    WOL = nc.dram_tensor("WOL", [DLOC, D], _dt.float8e5, kind="ExternalInput")
    COS = nc.dram_tensor("COS", [P, T], _dt.bfloat16, kind="ExternalInput")
    SIN = nc.dram_tensor("SIN", [P, T], _dt.bfloat16, kind="ExternalInput")
    ROT = nc.dram_tensor("ROT", [P, P], _dt.bfloat16, kind="ExternalInput")
    ONES = nc.dram_tensor("ONES", [P, P], _dt.bfloat16, kind="ExternalInput")
    IDN = nc.dram_tensor("IDN", [P, P], _dt.bfloat16, kind="ExternalInput")
    MD = nc.dram_tensor("MD", [P, 4, 512], _dt.bfloat16, kind="ExternalInput")
    # bf16 partials: halves the output DMA; host accumulates in fp32
    yT = nc.dram_tensor("yT", [D, TOK], _dt.bfloat16, kind="ExternalOutput")
    if DEBUG_DUMP:
        DBG_DN = nc.dram_tensor("DBG_DN", [P, 4], _dt.float32,
                                kind="ExternalOutput")
        DBG_RCT = nc.dram_tensor("DBG_RCT", [1, 4, P], _dt.bfloat16,
                                 kind="ExternalOutput")
        DBG_RBS = nc.dram_tensor("DBG_RBS", [P, 512], _dt.bfloat16,
                                 kind="ExternalOutput")
        DBG_T3 = nc.dram_tensor("DBG_T3", [P, 512], _dt.bfloat16,
                                kind="ExternalOutput")
        DBG_OP = nc.dram_tensor("DBG_OP", [P, 512], _dt.float32,
                                kind="ExternalOutput")

    xh_r = XH.ap().rearrange("(ko p) m -> p ko m", p=P)    # [128, 16, 4096]
    xl_r = XL.ap().rearrange("(ko p) m -> p ko m", p=P)
    wqh_r = WQH.ap().rearrange("(ko p) n -> p ko n", p=P)  # [128, 16, 256]
    wql_r = WQL.ap().rearrange("(ko p) n -> p ko n", p=P)
    wkh_r = WKH.ap().rearrange("(ko p) n -> p ko n", p=P)
    wkl_r = WKL.ap().rearrange("(ko p) n -> p ko n", p=P)
    wvh_r = WVH.ap().rearrange("(ko p) n -> p ko n", p=P)
    wvl_r = WVL.ap().rearrange("(ko p) n -> p ko n", p=P)
    woh_r = WOH.ap().rearrange("(ho p) n -> p ho n", p=P)  # [128, 2, 2048]
    wol_r = WOL.ap().rearrange("(ho p) n -> p ho n", p=P)

    with tile.TileContext(nc) as tc:
        with (
            tc.tile_pool(name="const", bufs=1) as cp,
            tc.tile_pool(name="data", bufs=1) as dp,
            tc.tile_pool(name="xs", bufs=2) as xp,
            tc.tile_pool(name="work", bufs=3) as wp,
        ):
            wqh_sb = cp.tile([P, KD, DLOC], _dt.float8e4, tag="wqh")
            wql_sb = cp.tile([P, KD, DLOC], _dt.float8e5, tag="wql")
            wkh_sb = cp.tile([P, KD, DLOC], _dt.float8e4, tag="wkh")
            wkl_sb = cp.tile([P, KD, DLOC], _dt.float8e5, tag="wkl")
            wvh_sb = cp.tile([P, KD, DLOC], _dt.float8e4, tag="wvh")
            wvl_sb = cp.tile([P, KD, DLOC], _dt.float8e5, tag="wvl")
            woh_sb = cp.tile([P, HL, D], _dt.float8e4, tag="woh")
            wol_sb = cp.tile([P, HL, D], _dt.float8e5, tag="wol")
            cos_sb = cp.tile([P, T], _dt.bfloat16, tag="cos")
            sin_sb = cp.tile([P, T], _dt.bfloat16, tag="sin")
            rot_sb = cp.tile([P, P], _dt.bfloat16, tag="rot")
            ones_sb = cp.tile([P, P], _dt.bfloat16, tag="ones")
            idn_sb = cp.tile([P, P], _dt.bfloat16, tag="idn")
            md_sb = cp.tile([P, 4, 512], _dt.bfloat16, tag="md")

            # persistent activations (partition = head-dim except v_sb);
            # RoPE is applied in place, so qt/kt double as qr/kr.
            qt_sb = dp.tile([P, HL, TOK], _dt.bfloat16, tag="qt")
            kt_sb = dp.tile([P, HL, TOK], _dt.bfloat16, tag="kt")
            qr_sb = qt_sb
            kr_sb = kt_sb
            v_sb = dp.tile([P, TOK // P, DLOC], _dt.bfloat16, tag="v")

            # ------- phase A: QKV projections with RoPE interleaved -------
            with tc.tile_pool(name="psproj", bufs=1, space="PSUM") as pp, \
                 tc.tile_pool(name="psv", bufs=2, space="PSUM") as pv, \
                 tc.tile_pool(name="psrot", bufs=2, space="PSUM") as pr:
                for nt in range(NT):
                    ts0 = nt * 512
                    xh_ts = xp.tile([P, KD, 512], _dt.float8e4, tag="xh")
                    xl_ts = xp.tile([P, KD, 512], _dt.float8e5, tag="xl")
                    if nt == 0:
                        # chunked first tile + interleaved one-time weight
                        # loads so the first matmuls start within a few us
                        # arrival order matched to the Q->K->V chain
                        # consumption order to minimize startup stalls
                        for kc in range(0, KD, 4):
                            nc.sync.dma_start(xh_ts[:, kc:kc + 4, :],
                                              xh_r[:, kc:kc + 4, ts0:ts0 + 512])
                            nc.sync.dma_start(wqh_sb[:, kc:kc + 4, :],
                                              wqh_r[:, kc:kc + 4, :])
                        for kc in range(0, KD, 4):
                            nc.sync.dma_start(xl_ts[:, kc:kc + 4, :],
                                              xl_r[:, kc:kc + 4, ts0:ts0 + 512])
                            nc.sync.dma_start(wql_sb[:, kc:kc + 4, :],
                                              wql_r[:, kc:kc + 4, :])
                        nc.sync.dma_start(wkh_sb[:], wkh_r)
                        nc.sync.dma_start(wkl_sb[:], wkl_r)
                        nc.sync.dma_start(wvh_sb[:], wvh_r)
                        nc.sync.dma_start(wvl_sb[:], wvl_r)
                        # must be emitted before their first readers (the
                        # nt=0 RoPE) -- dep tracking is program-order
                        nc.sync.dma_start(cos_sb[:], COS[:])
                        nc.sync.dma_start(sin_sb[:], SIN[:])
                        nc.sync.dma_start(rot_sb[:], ROT[:])
                    else:
                        nc.sync.dma_start(xh_ts[:], xh_r[:, :, ts0:ts0 + 512])
                        nc.sync.dma_start(xl_ts[:], xl_r[:, :, ts0:ts0 + 512])
                        if nt == 1:
                            nc.sync.dma_start(ones_sb[:], ONES[:])
                            nc.sync.dma_start(idn_sb[:], IDN[:])
                            nc.sync.dma_start(md_sb[:], MD[:])
                            nc.sync.dma_start(woh_sb[:], woh_r)
                            nc.sync.dma_start(wol_sb[:], wol_r)
                    psQ = pp.tile([P, HL, 512], _dt.float32, tag="psQ")
                    psK = pp.tile([P, HL, 512], _dt.float32, tag="psK")
                    # 3-term fp8 DoubleRow accumulation chains (Q fully
                    # before K: matches the nt==0 weight arrival order)
                    for ps, wh, wl in ((psQ, wqh_sb, wql_sb),
                                       (psK, wkh_sb, wkl_sb)):
                        for m in range(HL):
                            ms = slice(m * P, (m + 1) * P)
                            for j in range(KD // 2):
                                js = slice(2 * j, 2 * j + 2)
                                nc.tensor.matmul(ps[:, m, :], wh[:, js, ms],
                                                 xh_ts[:, js, :],
                                                 start=(j == 0), stop=False,
                                                 perf_mode=DR)
                            for j in range(KD // 2):
                                js = slice(2 * j, 2 * j + 2)
                                nc.tensor.matmul(ps[:, m, :], wh[:, js, ms],
                                                 xl_ts[:, js, :],
                                                 start=False, stop=False,
                                                 perf_mode=DR)
                            for j in range(KD // 2):
                                js = slice(2 * j, 2 * j + 2)
                                nc.tensor.matmul(ps[:, m, :], wl[:, js, ms],
                                                 xh_ts[:, js, :],
                                                 start=False,
                                                 stop=(j == KD // 2 - 1),
                                                 perf_mode=DR)
                    # V in natural layout: one PSUM bank per token block
                    for tb in range(4):
                        tbs = slice(tb * P, (tb + 1) * P)
                        psv = pv.tile([P, DLOC], _dt.float32, tag="psV")
                        for j in range(KD // 2):
                            js = slice(2 * j, 2 * j + 2)
                            nc.tensor.matmul(psv[:], xh_ts[:, js, tbs],
                                             wvh_sb[:, js, :],
                                             start=(j == 0), stop=False,
                                             perf_mode=DR)
                        for j in range(KD // 2):
                            js = slice(2 * j, 2 * j + 2)
                            nc.tensor.matmul(psv[:], xh_ts[:, js, tbs],
                                             wvl_sb[:, js, :],
                                             start=False, stop=False,
                                             perf_mode=DR)
                        for j in range(KD // 2):
                            js = slice(2 * j, 2 * j + 2)
                            nc.tensor.matmul(psv[:], xl_ts[:, js, tbs],
                                             wvh_sb[:, js, :],
                                             start=False,
                                             stop=(j == KD // 2 - 1),
                                             perf_mode=DR)
                        nc.scalar.copy(v_sb[:, nt * 4 + tb, :], psv[:])
                    # RoPE for this token tile; psum->sbuf copies on ACT,
                    # cos-mul on DVE (4x bf16 mode), combine-add on DVE
                    c0 = (nt % QT_PER_B) * 512
                    for ps, dst in ((psQ, qt_sb), (psK, kt_sb)):
                        for m in range(HL):
                            sl = dst[:, m, ts0:ts0 + 512]
                            nc.scalar.copy(sl, ps[:, m, :])
                            rp = pr.tile([P, 512], _dt.float32, tag="rot")
                            nc.tensor.matmul(rp[:], rot_sb[:], sl,
                                             start=True, stop=True)
                            t1 = wp.tile([P, 512], _dt.bfloat16, tag="t1")
                            nc.vector.tensor_mul(t1[:], sl,
                                                 cos_sb[:, c0:c0 + 512])
                            t2 = wp.tile([P, 512], _dt.bfloat16, tag="t2")
                            nc.vector.tensor_mul(t2[:], rp[:],
                                                 sin_sb[:, c0:c0 + 512])
                            # all-SBUF bf16 add: offload to the idle gpsimd
                            nc.gpsimd.tensor_add(sl, t1[:], t2[:])

            # ------- phase B: attention with output proj interleaved -------
            # The output projection of query tile N is emitted DURING tile
            # N+1's attention, in four groups placed exactly where the
            # softmax-denominator chain (reciprocal -> transpose -> rcT copy
            # -> broadcast) would otherwise leave the in-order PE waiting on
            # DVE results.
            with tc.tile_pool(name="psatt", bufs=2, space="PSUM") as pa, \
                 tc.tile_pool(name="psy", bufs=2, space="PSUM") as py:

                def make_wo(q0, ot8h, ot8l, tail):
                    def emit_wo(g0, g1):
                        for nbg in range(g0, g1):
                            ysb = wp.tile([P, 4, 512], _dt.bfloat16,
                                          tag="ysb", bufs=3)
                            for i in range(4):
                                nb = nbg * 4 + i
                                nbs = slice(nb * P, (nb + 1) * P)
                                yp = py.tile([P, 512], _dt.float32, tag="y")
                                nc.tensor.matmul(yp[:], woh_sb[:, :, nbs],
                                                 ot8h[:], start=True,
                                                 stop=False, perf_mode=DR)
                                nc.tensor.matmul(yp[:], woh_sb[:, :, nbs],
                                                 ot8l[:], start=False,
                                                 stop=False, perf_mode=DR)
                                nc.tensor.matmul(yp[:], wol_sb[:, :, nbs],
                                                 ot8h[:], start=False,
                                                 stop=True, perf_mode=DR)
                                if i % 2 == 0:
                                    nc.vector.tensor_copy(ysb[:, i, :], yp[:])
                                else:
                                    nc.scalar.copy(ysb[:, i, :], yp[:])
                            nc.sync.dma_start(
                                yT[nbg * 512:(nbg + 1) * 512, q0:q0 + 512]
                                .rearrange("(i p) q -> p i q", p=P), ysb[:])
                    return emit_wo

                pending_wo = None
                for b in range(B):
                    # descending qt: the cheapest attention tile runs last,
                    # shortening the non-overlapped kernel tail
                    for qt in reversed(range(QT_PER_B)):
                        q0 = b * T + qt * 512
                        nk = (qt + 1) * 4
                        ot8h = wp.tile([P, HL, 512], _dt.float8e4,
                                       tag="ot8h", bufs=2,
                                       name=f"ot8h_{b}_{qt}")
                        ot8l = wp.tile([P, HL, 512], _dt.float8e5,
                                       tag="ot8l", bufs=2,
                                       name=f"ot8l_{b}_{qt}")
                        for hl in range(HL):
                            op = pa.tile([P, 512], _dt.float32, tag="o",
                                         bufs=2)
                            dn = pa.tile([P, 4], _dt.float32, tag="dn",
                                         bufs=1)

                            # software-pipelined: emit tile kt's QK/exp one
                            # step ahead of tile kt-1's PV/dn so the PE
                            # priority order prefers independent matmuls
                            # while the exp is in flight (same math)
                            def emit_qk(kt):
                                j = kt - 4 * qt
                                qoff = max(j, 0) * P
                                nq = 512 - qoff
                                k0 = b * T + kt * P
                                sp_ = pa.tile([P, 512], _dt.float32, tag="s",
                                              bufs=3, name=f"s_{b}_{hl}_{kt}")
                                nc.tensor.matmul(
                                    sp_[:, :nq], kr_sb[:, hl, k0:k0 + P],
                                    qr_sb[:, hl, q0 + qoff:q0 + 512],
                                    start=True, stop=True)
                                pT = wp.tile([P, 512], _dt.bfloat16, tag="pT",
                                             bufs=6, name=f"p_{b}_{hl}_{kt}")
                                nc.scalar.activation(
                                    pT[:, :nq], sp_[:, :nq],
                                    mybir.ActivationFunctionType.Exp,
                                    scale=SCALE)
                                if j >= 0:  # 0/1 mask inside the diagonal
                                    nc.vector.tensor_mul(pT[:, :nq],
                                                         pT[:, :nq],
                                                         md_sb[:, j, qoff:])
                                return pT, qoff, nq

                            def emit_pv(kt, pT, qoff, nq):
                                j = kt - 4 * qt
                                st = (kt == 0)
                                sp2 = (kt == nk - 1)
                                nc.tensor.matmul(
                                    op[:, qoff:],
                                    v_sb[:, b * (T // P) + kt,
                                         hl * P:(hl + 1) * P],
                                    pT[:, :nq], start=st, stop=sp2)
                                # denominator: pT stationary, ones moving.
                                # A start=True matmul zeroes the WHOLE psum
                                # bank, so only the very first chunk write
                                # may carry it; the other kt==0 chunks land
                                # on pending-zero bytes and still overwrite.
                                # One stop on the last instr (diag j==3).
                                for c in range(qoff // P, 4):
                                    nc.tensor.matmul(
                                        dn[:, c:c + 1],
                                        pT[:, c * P - qoff:c * P - qoff + P],
                                        ones_sb[:, 0:1],
                                        start=(st and c == 0),
                                        stop=(j == 3 and c == 3),
                                        skip_group_check=True)

                            # two QK/exp tiles in flight ahead of each PV so
                            # the PE never waits on the ACT exp latency
                            fifo = [emit_qk(0), emit_qk(1)]
                            for kt in range(2, nk):
                                fifo.append(emit_qk(kt))
                                emit_pv(kt - 2, *fifo.pop(0))
                            emit_pv(nk - 2, *fifo.pop(0))
                            emit_pv(nk - 1, *fifo.pop(0))

                            # denominator reciprocal, then partition-major
                            # -> free-major reshuffle and broadcast entirely
                            # on DMA + the idle gpsimd queue; the previous
                            # query tile's Wo groups keep the PE fed while
                            # this chain is in flight
                            rcf = wp.tile([P, 4], _dt.float32, tag="rcf")
                            nc.vector.reciprocal(rcf[:], dn[:])
                            rcb = wp.tile([P, 4], _dt.bfloat16, tag="rcb")
                            nc.vector.tensor_copy(rcb[:], rcf[:])
                            rcTf = wp.tile([1, 512], _dt.bfloat16, tag="rcTf",
                                           bufs=2)
                            for c in range(4):
                                nc.sync.dma_start(
                                    rcTf[0:1, c * P:(c + 1) * P],
                                    rcb[:, c:c + 1])
                            rbS = wp.tile([P, 512], _dt.bfloat16, tag="rbS",
                                          bufs=2)
                            nc.gpsimd.partition_broadcast(rbS[:],
                                                          rcTf[0:1, :])
                            if pending_wo is not None:
                                pending_wo(2 * hl, 2 * hl + 2)
                            # normalized head output in fp8 hi/lo for the
                            # DoubleRow output projection
                            t3 = wp.tile([P, 512], _dt.bfloat16, tag="t3",
                                         bufs=2)
                            nc.vector.tensor_mul(t3[:], op[:], rbS[:])
                            nc.vector.tensor_copy(ot8h[:, hl, :], t3[:])
                            nc.vector.tensor_sub(ot8l[:, hl, :], t3[:],
                                                 ot8h[:, hl, :])
                            if DEBUG_DUMP and b == 0 and qt == 0 and hl == 0:
                                dbg_dn = wp.tile([P, 4], _dt.float32,
                                                 tag="dbgdn")
                                nc.vector.tensor_copy(dbg_dn[:], dn[:])
                                nc.sync.dma_start(DBG_DN.ap(), dbg_dn[:])
                                nc.sync.dma_start(DBG_RCT.ap(), rcT[:])
                                nc.sync.dma_start(DBG_RBS.ap(), rbS[:])
                                nc.sync.dma_start(DBG_T3.ap(), t3[:])
                                dbg_op = wp.tile([P, 512], _dt.float32,
                                                 tag="dbgop")
                                nc.vector.tensor_copy(dbg_op[:], op[:])
                                nc.sync.dma_start(DBG_OP.ap(), dbg_op[:])
                        pending_wo = make_wo(q0, ot8h, ot8l, tail=False)
                # flush the final query tile's output projection
                pending_wo(0, 4)

    nc.compile()
    return nc


_NC_CACHE = None


def _get_nc():
    global _NC_CACHE
    if _NC_CACHE is None:
        _NC_CACHE = _build_kernel()
    return _NC_CACHE


def _rope_tables():
    inv_freq = 1.0 / THETA ** (np.arange(0, DK, 2, dtype=np.float32) / DK)
    t = np.arange(T, dtype=np.float32)
    freqs = np.outer(t, inv_freq)                 # (T, dk/2)
    freqs = np.repeat(freqs, 2, axis=-1)          # (T, dk)
    return np.cos(freqs), np.sin(freqs)


def _hi_lo(a):
    """fp32 -> (e4m3 hi, e5m2 lo) split, as contiguous arrays."""
    hi = np.ascontiguousarray(a).astype(E4)
    lo = (a - hi.astype(np.float32)).astype(E5)
    return hi, lo


def _host_inputs(x, Wq, Wk, Wv, Wo):
    """Build the per-core input maps (all host-side prep is free)."""
    xT = np.ascontiguousarray(x.reshape(TOK, D).T)   # [D, B*T] fp32
    xh, xl = _hi_lo(xT)
    cos, sin = _rope_tables()                        # (T, dk)
    cosT = np.ascontiguousarray(cos.T).astype(BF16)  # [128, T]
    sinT = np.ascontiguousarray(sin.T).astype(BF16)

    rot = np.zeros((P, P), dtype=np.float32)
    for i in range(P // 2):
        rot[2 * i + 1, 2 * i] = -1.0   # (R^T)[2i, 2i+1] = -1
        rot[2 * i, 2 * i + 1] = 1.0    # (R^T)[2i+1, 2i] = +1
    rot = rot.astype(BF16)
    ones = np.ones((P, P), dtype=BF16)
    idn = np.eye(P, dtype=BF16)

    # diagonal-block masks, scores layout [key, query]; offset j*128
    md = np.zeros((4, P, 512), dtype=np.float32)
    kk = np.arange(P)[:, None]
    qq = np.arange(512)[None, :]
    for j in range(4):
        md[j] = (qq >= kk + j * P).astype(np.float32)
    md = np.ascontiguousarray(md.transpose(1, 0, 2)).astype(BF16)

    in_maps = []
    for c in range(NCORES):
        rows = slice(c * DLOC, (c + 1) * DLOC)
        wqh, wql = _hi_lo(Wq[rows, :].T)
        wkh, wkl = _hi_lo(Wk[rows, :].T)
        wvh, wvl = _hi_lo(Wv[rows, :].T)
        woh, wol = _hi_lo(Wo[:, rows].T)
        in_maps.append({
            "XH": xh, "XL": xl,
            "WQH": wqh, "WQL": wql, "WKH": wkh, "WKL": wkl,
            "WVH": wvh, "WVL": wvl, "WOH": woh, "WOL": wol,
            "COS": cosT, "SIN": sinT, "ROT": rot, "ONES": ones, "IDN": idn,
            "MD": md,
        })
    return in_maps


def _run(in_maps, **kwargs):
    nc = _get_nc()
    return run_bass_kernel_spmd(nc, in_maps, core_ids=list(range(NCORES)),
                                **kwargs)


def kernel(x, Wq, Wk, Wv, Wo, mask, _bench_results=None, **_kw):
    x = np.asarray(x, dtype=np.float32)
    Wq = np.asarray(Wq, dtype=np.float32)
    Wk = np.asarray(Wk, dtype=np.float32)
    Wv = np.asarray(Wv, dtype=np.float32)
    Wo = np.asarray(Wo, dtype=np.float32)
    mask = np.asarray(mask)
    causal = np.array_equal(mask.reshape(T, T),
                            np.tril(np.ones((T, T), dtype=bool)))
    if not causal:
        raise NotImplementedError("kernel specialized for the causal mask")

    res = _run(_host_inputs(x, Wq, Wk, Wv, Wo))
    if _bench_results is not None:
        _bench_results.append(res)

    acc = np.zeros((D, TOK), dtype=np.float32)
    for r in res.results:
        acc += r["yT"].astype(np.float32)
    # yT[n, b*T + t] -> out[b, t, n]
    return np.ascontiguousarray(acc.reshape(D, B, T).transpose(1, 2, 0))
